# revision 34
# baseline (speedup 1.0000x reference)
"""Trainium2 Bass kernel for nn_DenseOrderOneTerm (B=16384, D=32, H=512, 8 cores).

new_q = expm(dt*clip(MLP(p,t),±20)) @ q ; dlogp = dt*trace(clip(MLP(p,t),±20))

Strategy (per core, 2048 samples):
 - 5-layer time-injection MLP on the PE in fp16 (fp32 PSUM accumulate).
   Layers 0-3 feature-major [features, batch]; output layer flips to
   batch-major by making the activations the stationary operand.
   Biases + t-feature folded into extra contraction rows host-side.
 - SELU composed from Relu/Exp on ACT + two fused DVE ops; the selu
   lambda scale and the -lambda*alpha offset are folded into the next
   layer's weights/biases host-side.
 - Matrix exponential action: new_q = sum_k c_k (D/rho)^k q, computed as
   batched matvecs in batch-major layout: one DVE broadcast-multiply per
   term (fp16, 2x mode) + a log2 tree of TT adds for the grouped j-sum
   (tensor_reduce only runs at 1x). Chunks are split into two groups
   whose trees run on DVE and GpSimd so the serial per-term chains
   overlap engines. Degree/substeps picked from dt at build time; Wout
   pre-scaled by 1/rho for fp16 dynamic range of the Krylov iterates.
"""

import math

import numpy as np
import ml_dtypes

import concourse.bass as bass
import concourse.tile as tile
import concourse.mybir as mybir
from concourse.bass_utils import run_bass_kernel_spmd

B, D, H = 16384, 32, 512
N_CORES = 8
BL = B // N_CORES          # 2048 samples per core
NT = BL // 128             # 16 batch tiles per core
LAM = 1.0507009873554805
ALPHA = 1.6732632423543772
RHO = 4.5                  # safe bound on spectral norm of dense (measured max ~2.2)
CLIP = 20.0
F16 = mybir.dt.float16
F32 = mybir.dt.float32


def _pick_schedule(dt):
    theta = abs(dt) * RHO
    S = max(1, int(math.ceil(theta / 1.2)))
    th = theta / S
    d = 1
    while th ** (d + 1) / math.factorial(d + 1) > 2.5e-4 and d < 40:
        d += 1
    return S, d


def _split_wide_waits(nc):
    """This walrus rejects >1 semaphore wait on most instructions: move
    excess waits onto preceding same-engine Drain instructions."""
    cnt = 0
    for f in nc.m.functions:
        for b in f.blocks:
            insts = b.instructions
            new_list = []
            changed = False
            for inst in insts:
                si = inst.sync_info
                waits = list(si.on_wait) if si is not None and si.on_wait else []
                if len(waits) > 1:
                    for w in waits[:-1]:
                        dr = mybir.InstDrain(name=f"wsplit-{cnt}", ins=[], outs=[])
                        cnt += 1
                        dr.engine = inst.engine
                        dr.sync_info = mybir.SyncInfo(on_wait=[w], on_update=[])
                        new_list.append(dr)
                    inst.sync_info = mybir.SyncInfo(
                        on_wait=[waits[-1]],
                        on_update=list(si.on_update) if si.on_update else [],
                    )
                    changed = True
                new_list.append(inst)
            if changed:
                insts.clear()
                insts.extend(new_list)
    return cnt


def _ap_sig(a):
    try:
        return (a.memref, a.offset, tuple(tuple(x) for x in a.ap), a.dtype)
    except Exception:
        return None


def _dedupe_ldweights(nc):
    """Remove InstLdweights identical to the immediately-preceding one on PE
    (weights stay resident in the array across matmuls). Waits/updates of a
    removed LDW migrate to the next kept instruction."""
    removed = 0
    for f in nc.m.functions:
        for b in f.blocks:
            insts = list(b.instructions)
            new_list = []
            last_sig = None
            pend_w, pend_u = [], []
            for inst in insts:
                nm = type(inst).__name__
                if nm == "InstLdweights":
                    sig = _ap_sig(inst.ins[0])
                    if sig is not None and sig == last_sig:
                        si = inst.sync_info
                        if si is not None:
                            pend_w.extend(list(si.on_wait or []))
                            pend_u.extend(list(si.on_update or []))
                        removed += 1
                        continue
                    last_sig = sig
                elif (str(inst.engine).endswith("PE")
                      and nm not in ("InstMatmult", "InstDrain",
                                     "InstEventSemaphore", "InstRegisterMove")):
                    last_sig = None  # unexpected PE op: be conservative
                if pend_w or pend_u:
                    si = inst.sync_info
                    w = list(si.on_wait or []) if si else []
                    u = list(si.on_update or []) if si else []
                    inst.sync_info = mybir.SyncInfo(
                        on_wait=pend_w + w, on_update=pend_u + u)
                    pend_w, pend_u = [], []
                new_list.append(inst)
            assert not pend_w and not pend_u
            if removed:
                ii = b.instructions
                ii.clear()
                ii.extend(new_list)
    return removed


def _build_program(dt, deg_override=None):
    S, DEG = _pick_schedule(dt)
    if deg_override is not None:
        DEG = deg_override
    dts = dt / S
    WS = 1.0 / RHO  # Wout pre-scale; dense_stored = clip(flat)/rho

    nc = bass.Bass("TRN2", target_bir_lowering=False, debug=False,
                   num_devices=N_CORES)

    # ---- DRAM I/O (per-core shapes) ----
    d_ptT = nc.dram_tensor("ptT", (34, BL), F16, kind="ExternalInput").ap()
    d_t1T = nc.dram_tensor("t1T", (3, BL), F16, kind="ExternalInput").ap()
    d_q16 = nc.dram_tensor("q16", (BL, D), F16, kind="ExternalInput").ap()
    d_qf = nc.dram_tensor("qf", (BL, D), F32, kind="ExternalInput").ap()
    d_W0 = nc.dram_tensor("W0", (34, H), F16, kind="ExternalInput").ap()
    d_Wm = [nc.dram_tensor(f"Wm{i}", (H, H), F16, kind="ExternalInput").ap()
            for i in range(1, 4)]
    d_We = [nc.dram_tensor(f"We{i}", (3, H), F16, kind="ExternalInput").ap()
            for i in range(1, 4)]
    d_Wom = nc.dram_tensor("Wom", (H, 2 * H), F16, kind="ExternalInput").ap()
    d_Woe = nc.dram_tensor("Woe", (3, 2 * H), F16, kind="ExternalInput").ap()
    d_newq = nc.dram_tensor("new_q", (BL, D), F32, kind="ExternalOutput").ap()
    d_dlogp = nc.dram_tensor("dlogp", (BL,), F32, kind="ExternalOutput").ap()

    FD = 1024  # D*D
    with tile.TileContext(nc) as tc:
        with (
            tc.tile_pool(name="wpool", bufs=1) as wp,
            tc.tile_pool(name="xpool", bufs=8) as xp,
            tc.tile_pool(name="tmp", bufs=8) as tp,
            tc.tile_pool(name="big", bufs=1) as bigp,
            tc.tile_pool(name="psA", bufs=3, space="PSUM") as psA,
            tc.tile_pool(name="psB", bufs=1, space="PSUM") as psB,
        ):
            # ---- weight / input loads ----
            w0 = wp.tile([128, H], F16, tag="w0")          # rows 0..33 used
            nc.sync.dma_start(w0[0:34, :], d_W0[:])
            wm = []
            we = []
            for i in range(3):
                t_ = wp.tile([128, 4 * H], F16, tag=f"wm{i}")
                nc.sync.dma_start(
                    t_[:].rearrange("p (c n) -> p c n", c=4),
                    d_Wm[i].rearrange("(c p) n -> p c n", p=128),
                )
                wm.append(t_)
                e_ = wp.tile([128, H], F16, tag=f"we{i}")   # rows 0..1 used
                nc.sync.dma_start(e_[0:3, :], d_We[i][:])
                we.append(e_)
            wom = wp.tile([128, 4 * 2 * H], F16, tag="wom")
            nc.sync.dma_start(
                wom[:].rearrange("p (c n) -> p c n", c=4),
                d_Wom.rearrange("(c p) n -> p c n", p=128),
            )
            woe = wp.tile([128, 2 * H], F16, tag="woe")     # rows 0..1 used
            nc.sync.dma_start(woe[0:3, :], d_Woe[:])

            lnb_c = wp.tile([128, 1], F32, tag="lnb")
            nc.vector.memset(lnb_c[:], math.log(LAM * ALPHA))

            ptT = wp.tile([128, BL], F16, tag="ptT")        # rows 0..33 used
            nc.sync.dma_start(ptT[0:34, :], d_ptT[:])
            t1T = wp.tile([128, BL], F16, tag="t1T")        # rows 0..1 used
            nc.sync.dma_start(t1T[0:3, :], d_t1T[:])

            # ---- MLP layers 0..3, feature-major x^T [512, BL] as 4x[128, BL]
            NB = BL // 512  # batch chunks of 512 for fp16 moving operand
            LNB = math.log(LAM * ALPHA)
            x = None
            for li in range(4):
                xn = [xp.tile([128, BL], F16, tag="x", name=f"x{li}_{h}") for h in range(4)]
                for h in range(4):
                    pss = []
                    for half in range(NB // 2):
                        ps = psA.tile([128, 1024], F32, tag="psA",
                                      name=f"ps{li}_{h}_{half}")
                        pss.append(ps)
                    for kc in range(5 if li else 1):
                        for nb in range(NB):
                            cs = slice(nb * 512, (nb + 1) * 512)
                            pso = pss[nb // 2][:, (nb % 2) * 512:(nb % 2 + 1) * 512]
                            if li == 0:
                                nc.tensor.matmul(
                                    pso, lhsT=w0[0:34, h * 128:(h + 1) * 128],
                                    rhs=ptT[0:34, cs], start=True, stop=True)
                            elif kc < 4:
                                nc.tensor.matmul(
                                    pso,
                                    lhsT=wm[li - 1][:, kc * H + h * 128:
                                                    kc * H + (h + 1) * 128],
                                    rhs=x[kc][:, cs], start=(kc == 0), stop=False)
                            else:
                                nc.tensor.matmul(
                                    pso,
                                    lhsT=we[li - 1][0:3, h * 128:(h + 1) * 128],
                                    rhs=t1T[0:3, cs], start=False, stop=True)
                    for half in range(NB // 2):
                        ps = pss[half]
                        cs = slice(half * 1024, (half + 1) * 1024)
                        # selu: x' = Relu(z) + exp(min(z,0)/lam + ln(lam*alpha))
                        # (z pre-scaled by lam; -lam*alpha folded into next bias)
                        m_ = tp.tile([128, 1024], F32, tag="m", name=f"m{li}{h}{half}")
                        nc.vector.tensor_scalar_min(m_[:], ps[:], 0.0)
                        r_ = tp.tile([128, 1024], F16, tag="r", name=f"r{li}{h}{half}")
                        nc.scalar.activation(
                            r_[:], ps[:], mybir.ActivationFunctionType.Relu)
                        e_ = tp.tile([128, 1024], F16, tag="e", name=f"e{li}{h}{half}")
                        nc.scalar.activation(
                            e_[:], m_[:], mybir.ActivationFunctionType.Exp,
                            scale=1.0 / LAM, bias=lnb_c[:])
                        nc.vector.tensor_tensor(
                            xn[h][:, cs], e_[:], r_[:], op=mybir.AluOpType.add)
                x = xn

            # ---- output layer: batch-major, activations stationary ----
            dense = bigp.tile([128, NT * FD], F16, tag="dense")
            dlp = bigp.tile([128, NT], F32, tag="dlp")
            CL = CLIP * WS
            for c in range(NT):
                bs = slice(c * 128, (c + 1) * 128)
                psb = psB.tile([128, 1024], F32, tag="psB", name=f"psB{c}")
                for kc in range(4):
                    for nf in range(2):
                        fs = slice(nf * 512, (nf + 1) * 512)
                        nc.tensor.matmul(
                            psb[:, fs], lhsT=x[kc][:, bs],
                            rhs=wom[:, kc * 2 * H:][:, fs],
                            start=(kc == 0), stop=False)
                for nf in range(2):
                    fs = slice(nf * 512, (nf + 1) * 512)
                    nc.tensor.matmul(
                        psb[:, fs], lhsT=t1T[0:3, bs], rhs=woe[0:3, fs],
                        start=False, stop=True)
                nc.vector.tensor_scalar(
                    dense[:, c * FD: (c + 1) * FD], psb[:], CL, -CL,
                    op0=mybir.AluOpType.min, op1=mybir.AluOpType.max)
                dg = tp.tile([128, 32], F32, tag="dg", name=f"dg{c}")
                nc.vector.tensor_scalar(
                    dg[:], psb[:, 0:1024:33][:, 0:32], CL, -CL,
                    op0=mybir.AluOpType.min, op1=mybir.AluOpType.max)
                nc.vector.tensor_reduce(
                    dlp[:, c:c + 1], dg[:], axis=mybir.AxisListType.X,
                    op=mybir.AluOpType.add)

            # dlogp = dt/WS * trace
            nc.scalar.mul(dlp[:], dlp[:], dt / WS)
            nc.sync.dma_start(d_dlogp.rearrange("(c p) -> p c", p=128), dlp[:])

            # ---- Krylov exp action ----
            y = bigp.tile([128, NT * D], F32, tag="y")
            nc.sync.dma_start(
                y[:].rearrange("p (c j) -> p c j", c=NT),
                d_qf.rearrange("(c p) j -> p c j", p=128))
            va = bigp.tile([128, NT * D], F16, tag="va")
            vb = bigp.tile([128, NT * D], F16, tag="vb")
            nc.sync.dma_start(
                va[:].rearrange("p (c j) -> p c j", c=NT),
                d_q16.rearrange("(c p) j -> p c j", p=128))

            r_ = bigp.tile([128, NT * FD], F16, tag="R")
            # two independent chunk groups: tree on DVE for G1, GpSimd for G2,
            # so the serial per-term chains of the two groups overlap engines
            NG1 = 12
            groups = []
            for (lo, hi, eng_is_gps) in ((0, NG1, False), (NG1, NT, True)):
                n = hi - lo
                r4 = r_[:, lo * FD:hi * FD].rearrange(
                    "p (c i j) -> p c i j", c=n, i=32)
                d4 = dense[:, lo * FD:hi * FD].rearrange(
                    "p (c i j) -> p c i j", c=n, i=32)
                groups.append((lo, hi, n, r4, d4, eng_is_gps))
            for s in range(S):
                if s > 0:
                    nc.vector.tensor_copy(va[:], y[:])
                ck = 1.0
                for k in range(1, DEG + 1):
                    ck *= dts * RHO / k
                    vin, vout = (va, vb) if k % 2 == 1 else (vb, va)
                    for (lo, hi, n, r4, d4, gps) in groups:
                        vbc = vin[:, lo * D:hi * D].rearrange(
                            "p (c j) -> p c j", c=n).unsqueeze(2).broadcast_to(
                            (128, n, 32, 32))
                        nc.vector.tensor_tensor(
                            r4, d4, vbc, op=mybir.AluOpType.mult)
                        eng = nc.gpsimd if gps else nc.vector
                        # log2 tree sum over j (TT adds at 2x; tensor_reduce is 1x)
                        for w in (16, 8, 4, 2):
                            eng.tensor_tensor(
                                r4[:, :, :, 0:w], r4[:, :, :, 0:w],
                                r4[:, :, :, w:2 * w], op=mybir.AluOpType.add)
                        eng.tensor_tensor(
                            vout[:, lo * D:hi * D].rearrange(
                                "p (c i) -> p c i", c=n).unsqueeze(3),
                            r4[:, :, :, 0:1], r4[:, :, :, 1:2],
                            op=mybir.AluOpType.add)
                        nc.vector.scalar_tensor_tensor(
                            y[:, lo * D:hi * D], vout[:, lo * D:hi * D], ck,
                            y[:, lo * D:hi * D],
                            op0=mybir.AluOpType.mult, op1=mybir.AluOpType.add)

            nc.sync.dma_start(
                d_newq.rearrange("(c p) j -> p c j", p=128),
                y[:].rearrange("p (c j) -> p c j", c=NT))

    _dedupe_ldweights(nc)
    _split_wide_waits(nc)
    return nc


_CACHE = {}


def _get_program(dt):
    key = round(float(dt), 9)
    if key not in _CACHE:
        _CACHE[key] = _build_program(float(dt))
    return _CACHE[key]


def _prep_host(q, p, t, dt, W0, b0, W1, b1, W2, b2, W3, b3, Wout, bout):
    f16 = np.float16
    WS = 1.0 / RHO
    Ws = [np.asarray(w, np.float64) for w in (W0, W1, W2, W3, Wout)]
    bs = [np.asarray(b_, np.float64) for b_ in (b0, b1, b2, b3, bout)]
    # fold the selu "+lam*alpha" constant offset of the previous layer's
    # activations into this layer's bias: b_adj = b - lam*alpha*colsum(W[:512])
    badj = [bs[0]]
    for i in range(1, 5):
        badj.append(bs[i] - LAM * ALPHA * Ws[i][:H, :].sum(axis=0))
    # layers 0..3 pre-scaled by lam (selu lambda)
    shared = {}
    W0f = np.concatenate([Ws[0], badj[0][None, :]], axis=0) * LAM  # [34, H]
    shared["W0"] = W0f.astype(f16)
    def hilo(v):
        hi = v.astype(f16).astype(np.float64)
        lo = v - hi
        return hi, lo

    for i in (1, 2, 3):
        shared[f"Wm{i}"] = (Ws[i][:H, :] * LAM).astype(f16)
        bh, bl = hilo(badj[i] * LAM)
        shared[f"We{i}"] = np.stack([Ws[i][H, :] * LAM, bh, bl]).astype(f16)
    shared["Wom"] = (Ws[4][:H, :] * WS).astype(f16)
    bh, bl = hilo(badj[4] * WS)
    shared["Woe"] = np.stack([Ws[4][H, :] * WS, bh, bl]).astype(f16)

    q = np.asarray(q, np.float32)
    p = np.asarray(p, np.float32)
    t = np.asarray(t, np.float32)
    in_maps = []
    for c in range(N_CORES):
        sl = slice(c * BL, (c + 1) * BL)
        pc, tc_, qc = p[sl], t[sl], q[sl]
        ptT = np.concatenate(
            [pc.T, tc_.T, np.ones((1, BL), np.float32)], axis=0)  # [34, BL]
        t1T = np.concatenate([tc_.T, np.ones((2, BL), np.float32)], axis=0)
        m = dict(shared)
        m["ptT"] = ptT.astype(f16)
        m["t1T"] = t1T.astype(f16)
        m["q16"] = qc.astype(f16)
        m["qf"] = qc
        in_maps.append(m)
    return in_maps


def kernel(q, p, t, dt, W0, b0, W1, b1, W2, b2, W3, b3, Wout, bout,
           _want_trace=False):
    nc = _get_program(float(dt))
    in_maps = _prep_host(q, p, t, dt, W0, b0, W1, b1, W2, b2, W3, b3,
                         Wout, bout)
    res = run_bass_kernel_spmd(nc, in_maps, core_ids=list(range(N_CORES)),
                               trace=_want_trace)
    new_q = np.concatenate([r["new_q"] for r in res.results], axis=0)
    dlogp = np.concatenate([r["dlogp"] for r in res.results], axis=0)
    if _want_trace:
        kernel._last_result = res
    return new_q, dlogp
